# revision 2
# baseline (speedup 1.0000x reference)
"""Trainium2 Bass kernel for nn_CinST_weight_59304908423199 (v2).

Same algorithm as v1 (bf16 compute, banded PE convs, DVE pools) with:
  - g-paired tiles x2[t] = [128, 2, 3200] so big DVE ops cover both c-groups
    in one instruction (halves DVE instruction count / init overhead).
  - pool h/w sums batched across t-pairs with a merged [128, 25, 112] mask
    (spw|sph): 800 -> 200 PE matmuls, results gathered via one [128, 8, 128]
    back-transpose per c-group.
  - t-sum / t-max / trees / final adds issued g-paired; conv grids paired.
  - late-loaded band constants reuse dead ycb/scratch SBUF slots.

Sharding: data-parallel over the 8 clips (batch dim), one clip per NeuronCore.
"""
import numpy as np
import ml_dtypes

bf16 = ml_dtypes.bfloat16

T, C, H, W = 8, 256, 56, 56
FLAT = H * W            # 3136
FLATP = 3200            # 25 * 128
NCH = 25
EPS = 1e-5
NCORES = 8


def _patch_tile_drain():
    """This walrus build only accepts one sync-wait per CTRL (Drain) instruction:
    spread the TileContext final-drain waits across multiple drains."""
    import concourse.tile as tile
    from concourse.vector_clock import ScopedClock

    if getattr(tile.TileContext, "_drain_patched", False):
        return

    def _drain_and_barrier(self, tick_clock, wait_clock):
        nc = self.nc
        drain_inst = nc.sync.drain()
        wait_clock.add_sem_waits(
            drain_inst.ins, ScopedClock({None: tick_clock.global_clock})
        )
        si = drain_inst.ins.sync_info
        if si is not None and si.on_wait and len(si.on_wait) > 1:
            waits = list(si.on_wait)
            si.on_wait = waits[:1]
            for wv in waits[1:]:
                d2 = nc.sync.drain()
                si2 = d2.ins.sync_info
                if si2 is None:
                    from concourse import mybir
                    d2.ins.sync_info = mybir.SyncInfo(on_wait=[wv], on_update=[])
                else:
                    si2.on_wait = [wv]
        nc.all_engine_barrier()
        assert self.sems is not None
        popped = nc._tile_sem_poison_stack.pop()
        assert popped is self._sem_poison
        nc.clear_and_free_semaphores(list(self.sems.allocated().values()))
        nc.all_engine_barrier()

    tile.TileContext._drain_and_barrier = _drain_and_barrier
    tile.TileContext._drain_patched = True


def make_consts(Wc, Wt, Wh, Ww):
    def band(vals, n):
        b = np.zeros((n, n), np.float32)
        for d, v in ((-1, vals[0]), (0, vals[1]), (1, vals[2])):
            idx = np.arange(max(0, -d), min(n, n - d))
            b[idx + d, idx] = v
        return b

    def bands18(Wm, axis, n, msc):
        out = np.zeros((2, 3, 3, n, n), np.float32)
        for ch in range(2):
            s = msc if ch == 1 else 1.0
            for a in range(3):
                for b_ in range(3):
                    if axis == 0:
                        vals = Wm[ch, :, a, b_]
                    elif axis == 1:
                        vals = Wm[ch, a, :, b_]
                    else:
                        vals = Wm[ch, a, b_, :]
                    out[ch, a, b_] = band(vals * s, n)
        return out.reshape(18, n, n).astype(bf16)

    winC = np.zeros((128, 119), np.float32)
    winC[:, 55] = 1.0
    oneh = np.zeros((2, 128), np.float32)
    oneh[0, 127] = 1.0
    oneh[1, 0] = 1.0
    # flat-position selection masks for PE pool-sums on transposed xc tiles:
    # chunk ci covers flat f = 128*ci + p; cols 0:56 sum over h (group by w),
    # cols 56:112 sum over w (group by h)
    spwh = np.zeros((128, NCH, 112), np.float32)
    for ci in range(NCH):
        for p in range(128):
            f = 128 * ci + p
            if f < FLAT:
                spwh[p, ci, f % 56] = 1.0
                spwh[p, ci, 56 + f // 56] = 1.0
    # row-selecting one-hots for the Wt corr matmul: the 129-wide last dim holds
    # 1s at cols 0 and 128 of row o; slicing cols [1:129] (g=0) yields the one-hot
    # at col 127, slicing [0:128] (g=1) yields it at col 0
    onehT = np.zeros((7, 7, 129), np.float32)
    for o in range(7):
        onehT[o, o, 0] = 1.0
        onehT[o, o, 128] = 1.0
    return {
        "oneh": oneh.astype(bf16),
        "onehT": onehT.astype(bf16),
        "bandWc": bands18(Wc[0], 1, 56, 1.0 / C),    # band over h; shifts (dt, dw)
        "bandWt": bands18(Wt[0], 0, 128, 1.0 / T),   # band over c(D); shifts (dh, dw)
        "bandWh": bands18(Wh[0], 1, 128, 1.0 / H),   # band over c(H); shifts (dt, dw)
        "bandWw": bands18(Ww[0], 2, 128, 1.0 / W),   # band over c(W); shifts (dt, dh)
        "winC": winC.astype(bf16),
        "ident": np.eye(128, dtype=np.float32).astype(bf16),
        "spwh": spwh.astype(bf16),
    }


def build_program(Wc, Wt, Wh, Ww, gamma, beta):
    import concourse.bass as bass
    import concourse.bass as _b
    import concourse.tile as tile
    from concourse import mybir

    _patch_tile_drain()

    F32, BF16 = mybir.dt.float32, mybir.dt.bfloat16
    AF = mybir.ActivationFunctionType
    ALU = mybir.AluOpType
    bnsc = [float(gamma[i]) / float(np.sqrt(1.0 + EPS)) for i in range(4)]
    bnbi = [float(beta[i]) for i in range(4)]
    WtE = [np.asarray(Wt[0, :, 2, :, :]), np.asarray(Wt[0, :, 0, :, :])]
    WhE = [np.asarray(Wh[0, :, :, 2, :]), np.asarray(Wh[0, :, :, 0, :])]
    WwE = [np.asarray(Ww[0, :, :, :, 2]), np.asarray(Ww[0, :, :, :, 0])]
    MSC = {"t": 1.0 / T, "h": 1.0 / H, "w": 1.0 / W}

    nc = bass.Bass("TRN2", target_bir_lowering=False, debug=False)
    x_in = nc.declare_dram_parameter("x", [T, C, H, W], F32, isOutput=False)
    out_d = nc.declare_dram_parameter("out", [T, C, H, W], F32, isOutput=True)
    cWc = nc.declare_dram_parameter("bandWc", [18, 56, 56], BF16, isOutput=False)
    cWt = nc.declare_dram_parameter("bandWt", [18, 128, 128], BF16, isOutput=False)
    cWh = nc.declare_dram_parameter("bandWh", [18, 128, 128], BF16, isOutput=False)
    cWw = nc.declare_dram_parameter("bandWw", [18, 128, 128], BF16, isOutput=False)
    cwinC = nc.declare_dram_parameter("winC", [128, 119], BF16, isOutput=False)
    coneh = nc.declare_dram_parameter("oneh", [2, 128], BF16, isOutput=False)
    cident = nc.declare_dram_parameter("ident", [128, 128], BF16, isOutput=False)
    conehT = nc.declare_dram_parameter("onehT", [7, 7, 129], BF16, isOutput=False)
    cspwh = nc.declare_dram_parameter("spwh", [128, NCH, 112], BF16, isOutput=False)

    def bc(base, freedims, extra=0):
        return bass.AP(tensor=base.tensor, offset=base.offset + extra,
                       ap=[base.ap[0]] + freedims)

    import contextlib
    with tile.TileContext(nc) as tc, contextlib.ExitStack() as ctx:
        singles = ctx.enter_context(tc.tile_pool(name="singles", bufs=1))
        bigp = ctx.enter_context(tc.tile_pool(name="big", bufs=1))
        trp = ctx.enter_context(tc.tile_pool(name="trp", bufs=1))
        ycbp = ctx.enter_context(tc.tile_pool(name="ycbp", bufs=2))
        scrp = ctx.enter_context(tc.tile_pool(name="scr", bufs=2))
        sgp = ctx.enter_context(tc.tile_pool(name="sgp", bufs=3))
        edgep = ctx.enter_context(tc.tile_pool(name="edgep", bufs=1))
        sbp = ctx.enter_context(tc.tile_pool(name="sbp", bufs=1))
        sm = ctx.enter_context(tc.tile_pool(name="sm", bufs=1))
        psp = ctx.enter_context(tc.tile_pool(name="ps", bufs=1, space="PSUM"))
        pscv = ctx.enter_context(tc.tile_pool(name="pscv", bufs=2, space="PSUM"))
        pssum = ctx.enter_context(tc.tile_pool(name="pssum", bufs=2, space="PSUM"))

        for _eng in (nc.vector, nc.scalar, nc.tensor, nc.gpsimd, nc.sync):
            _eng.nop(hint="wsplit_template")

        # ---- consts ----
        bWc = singles.tile([56, 18, 56], BF16, tag="bWc")
        nc.sync.dma_start(out=bWc[:], in_=cWc[:].rearrange("a b c -> b a c"))
        winC = singles.tile([128, 119], BF16, tag="winC")
        nc.sync.dma_start(out=winC[:], in_=cwinC[:])
        oneh127 = singles.tile([1, 128], BF16, tag="oneh127")
        nc.sync.dma_start(out=oneh127[:], in_=coneh[0:1, :])
        oneh0 = singles.tile([1, 128], BF16, tag="oneh0")
        nc.sync.dma_start(out=oneh0[:], in_=coneh[1:2, :])
        ident = singles.tile([128, 128], BF16, tag="ident")
        nc.sync.dma_start(out=ident[:], in_=cident[:])
        onehT = singles.tile([7, 7, 129], BF16, tag="onehT")
        nc.sync.dma_start(out=onehT[:], in_=conehT[:])
        SPWH = singles.tile([128, NCH, 112], BF16, tag="spwh")
        nc.sync.dma_start(out=SPWH[:], in_=cspwh[:])

        # ---- big tiles ----
        x2 = [bigp.tile([128, 2, FLATP], BF16, tag=f"x2_{t}", name=f"x2_{t}")
              for t in range(T)]
        tpa2 = [bigp.tile([128, 2, 3364], BF16, tag=f"tp2_{s}", name=f"tp2_{s}")
                for s in range(2)]
        hpa2 = [bigp.tile([128, 2, 580], BF16, tag=f"hp2_{s}", name=f"hp2_{s}")
                for s in range(2)]
        wpa2 = [bigp.tile([128, 2, 580], BF16, tag=f"wp2_{s}", name=f"wp2_{s}")
                for s in range(2)]
        sbAll = [sbp.tile([128, T, 128], BF16, tag=f"sba_{g}", name=f"sbAll_{g}")
                 for g in range(2)]
        sbT = [sbp.tile([128, T, 128], BF16, tag=f"sbt_{g}", name=f"sbT_{g}")
               for g in range(2)]
        cpmax_T = sm.tile([128, T, 32], BF16, tag="cpmaxT")
        cpmax_TT = sm.tile([128, 2, 128], BF16, tag="cpmaxTT")
        cp_main = sm.tile([56, 2, 10, 58], BF16, tag="cpmain")
        cpsum_r = sm.tile([64, 448], BF16, tag="cpsumr", name="cpsum_r")
        cpmax_f = sm.tile([8, FLATP], BF16, tag="cpmaxf")
        yc_flat = sm.tile([8, FLAT], BF16, tag="cpmaxf", name="yc_flat")

        # ---- border zeroing (interiors fully overwritten) ----
        for s in range(2):
            nc.vector.memset(bc(tpa2[s][:], [[3364, 2], [57 * 58, 2], [1, 58]]), 0.0)
            nc.vector.memset(bc(tpa2[s][:], [[3364, 2], [58, 58], [57, 2]]), 0.0)
            for grid in (hpa2[s], wpa2[s]):
                nc.vector.memset(bc(grid[:], [[580, 2], [9 * 58, 2], [1, 58]]), 0.0)
                nc.vector.memset(bc(grid[:], [[580, 2], [58, 10], [57, 2]]), 0.0)
        nc.vector.memset(bc(cpmax_T[:], [[32, T], [1, 32 - NCH]], NCH), 0.0)
        nc.gpsimd.memset(cp_main[:], 0.0)
        for g in range(2):
            nc.vector.memset(sbAll[g][:], 0.0)
        for t in range(T):
            nc.vector.memset(bc(x2[t][:], [[FLATP, 2], [1, FLATP - FLAT]], FLAT), 0.0)

        # ---- load (cast f32->bf16) ----
        for t in range(T):
            for g in range(2):
                nc.gpsimd.dma_start(
                    out=bc(x2[t][:], [[1, FLAT]], g * FLATP),
                    in_=x_in[t, g * 128:(g + 1) * 128, :, :].rearrange("c h w -> c (h w)"))

        # ---- P1: c-pool per t (premax, transpose, max tree; sum staircase) ----
        ps_cs = psp.tile([64, 448], F32, tag="ps_cs")
        for t in range(T):
            xm = scrp.tile([128, FLATP], BF16, tag="scratch", name=f"xm_{t}")
            nc.vector.tensor_tensor(out=xm[:], in0=bc(x2[t][:], [[1, FLATP]], 0),
                                    in1=bc(x2[t][:], [[1, FLATP]], FLATP), op=ALU.max)
            # xT rides the ycb ring: those 2 slots are idle until P2
            xT = ycbp.tile([128, NCH, 128], BF16, tag="ycb", name=f"xT_{t}")
            nc.sync.dma_start(out=xT[:], in_=xm[:], transpose=True)
            w_ = 64
            while w_ >= 2:
                nc.vector.tensor_tensor(out=xT[:, :, 0:w_], in0=xT[:, :, 0:w_],
                                        in1=xT[:, :, w_:2 * w_], op=ALU.max)
                w_ //= 2
            nc.vector.tensor_tensor(
                out=cpmax_T[:, t, 0:NCH],
                in0=bc(xT[:], [[128, NCH], [1, 1]], 0),
                in1=bc(xT[:], [[128, NCH], [1, 1]], 1), op=ALU.max)
            for o in range(7):
                r = t * 7 + o
                for g in range(2):
                    nc.tensor.matmul(
                        ps_cs[:], winC[:, 55 - r:119 - r],
                        bc(x2[t][:], [[1, 448]], g * FLATP + o * 448),
                        start=(r == 0 and g == 0), stop=(r == 55 and g == 1))
        # ---- assemble cp_main (max plane first: ready before the staircase) ----
        nc.sync.dma_start(out=cpmax_TT[:], in_=cpmax_T[:].rearrange("p a b -> p (a b)"),
                          transpose=True)
        for t in range(T):
            blk, row0 = divmod(t * 32, 128)
            # f-extraction on the scalar queue, grid row on sync: the two
            # per-t hops pipeline instead of serializing on one queue
            nc.scalar.dma_start(out=cpmax_f[t:t + 1, :], in_=cpmax_TT[row0:row0 + NCH, blk, :])
            nc.sync.dma_start(
                out=bc(cp_main[:], [[1, 56]], 0 + (t + 1) * 58 + 1),
                in_=cpmax_f[t:t + 1, 0:FLAT])
        nc.vector.tensor_copy(out=cpsum_r[:], in_=ps_cs[:])
        for t in range(T):
            nc.sync.dma_start(
                out=bc(cp_main[:], [[1, 56]], 580 + (t + 1) * 58 + 1),
                in_=cpsum_r[t * 7:(t + 1) * 7, :])

        # ---- Wc conv + sigmoid -> yc ----
        yc = sm.tile([56, T, W], BF16, tag="cpmain", name="yc")
        ps_yc = psp.tile([56, 448], F32, tag="ps_cs", name="ps_yc")
        k = 0
        for ch in (1, 0):  # sum plane first: its rows land before the max plane's
            for dt in range(3):
                for dw in range(3):
                    nc.tensor.matmul(
                        ps_yc[:], bWc[:, ch * 9 + dt * 3 + dw, :],
                        bc(cp_main[:], [[58, 8], [1, 56]], ch * 580 + dt * 58 + dw),
                        start=(k == 0), stop=(k == 17))
                    k += 1
        nc.scalar.activation(out=yc[:].rearrange("h t w -> h (t w)"), in_=ps_yc[:],
                             func=AF.Sigmoid, bias=bnbi[0], scale=bnsc[0])
        for t in range(T):
            nc.scalar.dma_start(out=yc_flat[t:t + 1, :], in_=yc[:, t, :])

        # ---- P2: xc, t-max, trees, pair-batched pool sums ----
        tmax_dst = bc(tpa2[0][:], [[3364, 2], [58, 56], [1, 56]], 59)
        for t in range(T):
            ycb = ycbp.tile([128, FLAT], BF16, tag="ycb", name=f"ycb_{t}")
            nc.sync.dma_start(
                out=ycb[:],
                in_=_b.AP(tensor=yc_flat[:].tensor, offset=yc_flat[:].offset + t * FLAT,
                          ap=[[FLAT, 1], [0, 128], [1, FLAT]]))
            xcv = bc(x2[t][:], [[FLATP, 2], [1, FLAT]])
            nc.vector.tensor_tensor(
                out=xcv, in0=xcv,
                in1=_b.AP(tensor=ycb[:].tensor, offset=ycb[:].offset,
                          ap=[ycb[:].ap[0], [0, 2], [1, FLAT]]),
                op=ALU.mult)
            # t-max running into the tpa2[0] grid interior
            x2v = bc(x2[t][:], [[FLATP, 2], [56, 56], [1, 56]])
            if t == 1:
                nc.vector.tensor_tensor(
                    out=tmax_dst, in0=bc(x2[0][:], [[FLATP, 2], [56, 56], [1, 56]]),
                    in1=x2v, op=ALU.max)
            elif t >= 2:
                nc.vector.tensor_tensor(out=tmax_dst, in0=tmax_dst, in1=x2v, op=ALU.max)
            # h-max tree (g-paired)
            hs = scrp.tile([128, 2, 28, 56], BF16, tag="scratch", name=f"hs_{t}")
            nc.vector.tensor_tensor(
                out=hs[:], in0=bc(x2[t][:], [[FLATP, 2], [56, 28], [1, 56]], 0),
                in1=bc(x2[t][:], [[FLATP, 2], [56, 28], [1, 56]], 28 * 56), op=ALU.max)
            n = 28
            while n > 1:
                h_ = n // 2
                nc.vector.tensor_tensor(out=hs[:, :, 0:h_, :], in0=hs[:, :, 0:h_, :],
                                        in1=hs[:, :, h_:2 * h_, :], op=ALU.max)
                if n % 2:
                    nc.vector.tensor_tensor(out=hs[:, :, 0:1, :], in0=hs[:, :, 0:1, :],
                                            in1=hs[:, :, n - 1:n, :], op=ALU.max)
                n = h_
            nc.vector.tensor_copy(
                out=bc(hpa2[0][:], [[580, 2], [1, 56]], (t + 1) * 58 + 1),
                in_=hs[:, :, 0, :])
            # w-max tree (g-paired)
            ws = scrp.tile([128, 2, 56, 28], BF16, tag="scratch", name=f"ws_{t}")
            nc.vector.tensor_tensor(
                out=ws[:], in0=bc(x2[t][:], [[FLATP, 2], [56, 56], [1, 28]], 0),
                in1=bc(x2[t][:], [[FLATP, 2], [56, 56], [1, 28]], 28), op=ALU.max)
            n = 28
            while n > 1:
                h_ = n // 2
                nc.vector.tensor_tensor(out=ws[:, :, :, 0:h_], in0=ws[:, :, :, 0:h_],
                                        in1=ws[:, :, :, h_:2 * h_], op=ALU.max)
                if n % 2:
                    nc.vector.tensor_tensor(out=ws[:, :, :, 0:1], in0=ws[:, :, :, 0:1],
                                            in1=ws[:, :, :, n - 1:n], op=ALU.max)
                n = h_
            nc.vector.tensor_copy(
                out=bc(wpa2[0][:], [[580, 2], [1, 56]], (t + 1) * 58 + 1),
                in_=ws[:, :, :, 0])
            # pair-batched pool sums on PE (at odd t)
            if t % 2 == 1:
                p = t // 2
                for g in range(2):
                    xq = trp.tile([128, 2, NCH, 128], BF16, tag="xq",
                                  name=f"xq_{p}_{g}")
                    for i in range(2):
                        nc.sync.dma_start(
                            out=xq[:, i, :, :],
                            in_=bc(x2[2 * p + i][:], [[1, FLATP]], g * FLATP),
                            transpose=True)
                    psq = pssum.tile([112, 2, 128], F32, tag="psq", name=f"psq_{p}_{g}")
                    for ci in range(NCH):
                        nc.tensor.matmul(
                            psq[:], SPWH[:, ci, :],
                            _b.AP(tensor=xq[:].tensor, offset=xq[:].offset + ci * 128,
                                  ap=[xq[:].ap[0], [NCH * 128, 2], [1, 128]]),
                            start=(ci == 0), stop=(ci == NCH - 1))
                    nc.scalar.activation(out=sbAll[g][0:112, 2 * p:2 * p + 2, :],
                                         in_=psq[:], func=AF.Copy)

        # ---- t-sum on PE (identity accumulate over t, per (g, o-chunk)) ----
        for g in range(2):
            for o in range(7):
                pts = pssum.tile([128, 448], F32, tag="pts", name=f"pts_{g}_{o}",
                                 bufs=3)
                for t in range(T):
                    nc.tensor.matmul(pts[:], ident[:],
                                     bc(x2[t][:], [[1, 448]], g * FLATP + o * 448),
                                     start=(t == 0), stop=(t == T - 1))
                nc.scalar.activation(
                    out=bc(tpa2[1][:], [[58, 8], [1, 56]], g * 3364 + (1 + 8 * o) * 58 + 1),
                    in_=bc(pts[:], [[56, 8], [1, 56]]), func=AF.Copy)

        # ---- back-transpose pool sums into c-partitioned grids ----
        for g in range(2):
            nc.sync.dma_start(out=sbT[g][:],
                              in_=sbAll[g][:].rearrange("p a b -> p (a b)"),
                              transpose=True)
            nc.vector.tensor_copy(
                out=bc(hpa2[1][:], [[58, 8], [1, 56]], g * 580 + 59),
                in_=sbT[g][:, :, 0:56])
            nc.vector.tensor_copy(
                out=bc(wpa2[1][:], [[58, 8], [1, 56]], g * 580 + 59),
                in_=sbT[g][:, :, 56:112])

        # ---- late consts into freed slots (ycb/scratch dead after P2) ----
        bWt = ycbp.tile([128, 18, 128], BF16, tag="ycb", name="bWt")
        nc.sync.dma_start(out=bWt[:], in_=cWt[:].rearrange("a b c -> b a c"))
        bWh = ycbp.tile([128, 18, 128], BF16, tag="ycb", name="bWh")
        nc.sync.dma_start(out=bWh[:], in_=cWh[:].rearrange("a b c -> b a c"))
        bWw = scrp.tile([128, 18, 128], BF16, tag="scratch", name="bWw")
        nc.sync.dma_start(out=bWw[:], in_=cWw[:].rearrange("a b c -> b a c"))

        # ---- boundary corrections for the c-banded convs ----
        # cfT aliases the dead cpmax_f/yc_flat slot; cfHW the dead cp_main/yc slot
        cfT = sm.tile([7, 2, 448], BF16, tag="cpmaxf", name="cfT")
        cfHW = sm.tile([1, 4, 448], BF16, tag="cpmain", name="cfHW")
        cfTv = [cfT[:, d, :] for d in range(2)]
        cfHv = [cfHW[:, d, :] for d in range(2)]
        cfWv = [cfHW[:, 2 + d, :] for d in range(2)]

        def make_corr(pools2, wE, kind, cfs, F):
            nr = 56 if kind == "t" else 8
            for d in range(2):
                src_g, src_p = (1, 0) if d == 0 else (0, 127)
                e3 = edgep.tile([58, 3, 2, 60], BF16, tag="edge", name=f"edge_{kind}_{d}")
                nc.vector.memset(e3[:], 0.0)
                for s_ in range(2):
                    base = pools2[s_][src_p:src_p + 1, :, :]
                    for a, (dst0, cnt, srcoff) in enumerate((
                            (1, nr, 0),         # e3[p] = field[p-1]
                            (0, nr, 0),         # e3[p] = field[p]
                            (0, nr - 1, 58))):  # e3[p] = field[p+1]
                        nc.sync.dma_start(
                            out=_b.AP(tensor=e3[:].tensor,
                                      offset=e3[:].offset + dst0 * 360 + a * 120 + s_ * 60 + 1,
                                      ap=[[360, cnt], [1, 56]]),
                            in_=_b.AP(tensor=base.tensor,
                                      offset=base.offset + src_g * F + 59 + srcoff,
                                      ap=[base.ap[0], [58, cnt], [1, 56]]))
                corr = edgep.tile([58, 56], BF16, tag="corr", name=f"corr_{kind}_{d}")
                nc.vector.memset(corr[:], 0.0)
                wm = wE[d]
                for ch in range(2):
                    sc = MSC[kind] if ch == 1 else 1.0
                    for a in range(3):
                        for b_ in range(3):
                            wv = float(wm[ch, a, b_]) * sc
                            nc.vector.scalar_tensor_tensor(
                                out=corr[0:nr, 0:56],
                                in0=_b.AP(tensor=e3[:].tensor,
                                          offset=e3[:].offset + a * 120 + ch * 60 + b_,
                                          ap=[[360, nr], [1, 56]]),
                                scalar=wv, in1=corr[0:nr, 0:56],
                                op0=ALU.mult, op1=ALU.add)
                nc.sync.dma_start(out=cfs[d], in_=corr[0:nr, 0:56])

        make_corr(hpa2, WhE, "h", cfHv, 580)
        make_corr(wpa2, WwE, "w", cfWv, 580)

        # ---- Wh conv -> yh3 (aliases hpa2[0] after its last conv read) ----
        sgh = []
        for g in range(2):
            ps = pscv.tile([128, 448], F32, tag="ps_cv", name="ps_h")
            k = 0
            for ch in range(2):
                for dt in range(3):
                    for db in range(3):
                        nc.tensor.matmul(
                            ps[:], bWh[:, ch * 9 + dt * 3 + db, :],
                            bc(hpa2[ch][:], [[58, 8], [1, 56]], g * 580 + dt * 58 + db),
                            start=(k == 0), stop=False)
                        k += 1
            nc.tensor.matmul(ps[:], oneh127 if g == 0 else oneh0,
                             cfHv[g], start=False, stop=True)
            sg = sgp.tile([128, 448], BF16, tag="sg", name=f"sg_h{g}")
            nc.scalar.activation(out=sg[:], in_=ps[:], func=AF.Sigmoid,
                                 bias=bnbi[2], scale=bnsc[2])
            sgh.append(sg)
        yh3 = bigp.tile([128, 2, T, W], BF16, tag="hp2_0", name="yh3")
        for g in range(2):
            nc.vector.tensor_scalar(out=bc(yh3[:], [[1, 448]], g * 448), in0=sgh[g][:],
                                    scalar1=1.0 / 3.0, scalar2=None, op0=ALU.mult)

        # ---- Ww conv -> yw3 (+ duplicated yw3d); alias dead hpa2[1]/xq slots ----
        sgw = []
        for g in range(2):
            ps = pscv.tile([128, 448], F32, tag="ps_cv", name="ps_w")
            k = 0
            for ch in range(2):
                for dt in range(3):
                    for db in range(3):
                        nc.tensor.matmul(
                            ps[:], bWw[:, ch * 9 + dt * 3 + db, :],
                            bc(wpa2[ch][:], [[58, 8], [1, 56]], g * 580 + dt * 58 + db),
                            start=(k == 0), stop=False)
                        k += 1
            nc.tensor.matmul(ps[:], oneh127 if g == 0 else oneh0,
                             cfWv[g], start=False, stop=True)
            sg = sgp.tile([128, 448], BF16, tag="sg", name=f"sg_w{g}")
            nc.scalar.activation(out=sg[:], in_=ps[:], func=AF.Sigmoid,
                                 bias=bnbi[3], scale=bnsc[3])
            sgw.append(sg)
        yw3 = bigp.tile([128, 2, T, H], BF16, tag="hp2_1", name="yw3")
        yw3d = trp.tile([128, 2, 448, 2], BF16, tag="xq", name="yw3d")
        for g in range(2):
            nc.vector.tensor_scalar(out=bc(yw3[:], [[1, 448]], g * 448), in0=sgw[g][:],
                                    scalar1=1.0 / 3.0, scalar2=None, op0=ALU.mult)
        # duplicate along trailing [2] so the final w-broadcast add keeps an
        # innermost stride-1 AP on every operand (DVE 2x mode)
        nc.scalar.activation(
            out=_b.AP(tensor=yw3d[:].tensor, offset=yw3d[:].offset,
                      ap=[yw3d[:].ap[0], [896, 2], [2, 448], [1, 2]]),
            in_=_b.AP(tensor=yw3[:].tensor, offset=yw3[:].offset,
                      ap=[yw3[:].ap[0], [448, 2], [1, 448], [0, 2]]),
            func=AF.Copy)

        # ---- Wt conv (banded over c) + sigmoid -> yt3 (aliases x2[0]);
        #      per-g: conv then adds+stores so g0's tail overlaps g1's conv ----
        make_corr(tpa2, WtE, "t", cfTv, 3364)
        yt3 = bigp.tile([128, 2, FLATP], BF16, tag="x2_0", name="yt3")
        v = [None] * T
        v0g = [None, None]
        v0g[0] = singles.tile([128, FLATP], BF16, tag="spwh", name="v_0_g0")
        v0g[1] = sm.tile([128, FLATP], BF16, tag="cpmaxf", name="v_0_g1")
        for t in range(1, T):
            v[t] = bigp.tile([128, 2, FLATP], BF16, tag=f"x2_{t}", name=f"v_{t}")
        def wt_block(g, o):
            ps = pscv.tile([128, 448], F32, tag="ps_cv", name="ps_wt")
            k = 0
            for ch in range(2):
                for dh in range(3):
                    for dw in range(3):
                        nc.tensor.matmul(
                            ps[:], bWt[:, ch * 9 + dh * 3 + dw, :],
                            bc(tpa2[ch][:], [[58, 8], [1, 56]],
                               g * 3364 + (8 * o + dh) * 58 + dw),
                            start=(k == 0), stop=False)
                        k += 1
            nc.tensor.matmul(
                ps[:],
                _b.AP(tensor=onehT[:].tensor,
                      offset=onehT[:].offset + o * 129 + (1 - g),
                      ap=[onehT[:].ap[0], [1, 128]]),
                cfTv[g], start=False, stop=True)
            sg = sgp.tile([128, 448], BF16, tag="sg", name=f"sg_t{g}{o}")
            nc.scalar.activation(out=sg[:], in_=ps[:], func=AF.Sigmoid,
                                 bias=bnbi[1], scale=bnsc[1])
            nc.vector.tensor_scalar(
                out=bc(yt3[:], [[1, 448]], g * FLATP + o * 448), in0=sg[:],
                scalar1=1.0 / 3.0, scalar2=None, op0=ALU.mult)

        def emit_add(g, t):
            # out = yt3 + yh3(bcast h) + yw3(bcast w), then cast-store
            vt = v0g[g][:] if t == 0 else bc(v[t][:], [[1, FLATP]], g * FLATP)
            nc.vector.tensor_tensor(
                out=bc(vt, [[1, FLAT]]),
                in0=bc(yt3[:], [[1, FLAT]], g * FLATP),
                in1=_b.AP(tensor=yh3[:].tensor,
                          offset=yh3[:].offset + g * 448 + t * 56,
                          ap=[yh3[:].ap[0], [0, 56], [1, 56]]),
                op=ALU.add)
            nc.vector.tensor_tensor(
                out=bc(vt, [[56, 56], [2, 28], [1, 2]]),
                in0=bc(vt, [[56, 56], [2, 28], [1, 2]]),
                in1=_b.AP(tensor=yw3d[:].tensor,
                          offset=yw3d[:].offset + g * 896 + t * 112,
                          ap=[yw3d[:].ap[0], [2, 56], [0, 28], [1, 2]]),
                op=ALU.add)
            nc.gpsimd.dma_start(
                out=out_d[t, g * 128:(g + 1) * 128, :, :].rearrange("c h w -> c (h w)"),
                in_=bc(vt, [[1, FLAT]]))

        for o in range(7):
            wt_block(0, o)
        for o in range(7):
            wt_block(1, o)
            emit_add(0, o)   # g0's adds+stores interleave with g1's conv blocks
        emit_add(0, 7)
        for t in range(T):
            emit_add(1, t)
    _split_multiwaits(nc, mybir)
    return nc


# Per-instruction sync-wait slot capacity of this walrus build (discovered
# empirically; excess waits are moved onto inserted same-engine nops).
WAIT_CAPS = {}
DEFAULT_WAIT_CAP = 1


def _split_multiwaits(nc, mybir):
    import copy
    templates = {}
    blocks = nc.m.functions[0].blocks
    for bb in blocks:
        for inst in bb.instructions:
            if type(inst).__name__ == "InstNoOp" and inst.engine not in templates:
                templates[inst.engine] = inst
    ctr = 0
    for bb in blocks:
        newl = []
        for inst in bb.instructions:
            si = getattr(inst, "sync_info", None)
            if si is not None and si.on_wait:
                cap = WAIT_CAPS.get(type(inst).__name__, DEFAULT_WAIT_CAP)
                waits = list(si.on_wait)
                if len(waits) > cap:
                    keep = waits[-cap:]
                    extra = waits[:-cap]
                    si.on_wait = keep
                    tpl = templates.get(inst.engine)
                    assert tpl is not None, f"no nop template for {inst.engine}"
                    nop_cap = WAIT_CAPS.get("InstNoOp", 1)
                    for i in range(0, len(extra), nop_cap):
                        nop = copy.deepcopy(tpl)
                        ctr += 1
                        nop.name = f"WSPLIT-{ctr}"
                        nop.sync_info = mybir.SyncInfo(
                            on_wait=extra[i:i + nop_cap], on_update=[])
                        newl.append(nop)
            newl.append(inst)
        if len(newl) != len(bb.instructions):
            bb.instructions[:] = newl
    return ctr


_CACHE = {}


def _get_program(Wc, Wt, Wh, Ww, gamma, beta):
    key = hash((Wc.tobytes(), Wt.tobytes(), Wh.tobytes(), Ww.tobytes(),
                gamma.tobytes(), beta.tobytes()))
    if key not in _CACHE:
        _CACHE[key] = (build_program(Wc, Wt, Wh, Ww, gamma, beta),
                       make_consts(Wc, Wt, Wh, Ww))
    return _CACHE[key]


def kernel(**inputs):
    x = np.ascontiguousarray(np.asarray(inputs["x"], dtype=np.float32))
    Wc = np.asarray(inputs["Wc"], dtype=np.float32)
    Wt = np.asarray(inputs["Wt"], dtype=np.float32)
    Wh = np.asarray(inputs["Wh"], dtype=np.float32)
    Ww = np.asarray(inputs["Ww"], dtype=np.float32)
    gamma = np.asarray(inputs["bn_gamma"], dtype=np.float32)
    beta = np.asarray(inputs["bn_beta"], dtype=np.float32)

    nc, consts = _get_program(Wc, Wt, Wh, Ww, gamma, beta)

    from concourse.bass_utils import run_bass_kernel_spmd

    in_maps = []
    for b in range(NCORES):
        m = {"x": x[b * T:(b + 1) * T].reshape(T, C, H, W)}
        m.update(consts)
        in_maps.append(m)
    res = run_bass_kernel_spmd(nc, in_maps, core_ids=list(range(NCORES)))
    out = np.empty((NCORES * T, C, H, W), np.float32)
    for b in range(NCORES):
        out[b * T:(b + 1) * T] = res.results[b]["out"]
    return out
